# revision 1
# baseline (speedup 1.0000x reference)
"""Trainium2 Bass kernel for nn_AttentionBranch: conv->relu->maxpool->conv->relu
followed by per-location rank-1 Gram outer products (100, 1024, 1024).

Sharding: the 100-location Gram axis is split across 8 NeuronCores
(13/12 locations per core). The conv backbone is replicated (conv1) /
channel-sliced to each core's needed 136-channel window (conv2), so no
collectives are required. The row-major .view(100, 1024) of the conv2
output is realised through a tiny DRAM scratch roundtrip.

Numerics: conv1 runs in fp32r (TensorE full-rate, ~2e-4), conv2 in exact
fp32, and the Gram products exactly in fp32 on VectorE/ScalarE
(tensor_scalar against a PE-broadcast row tile).

Output staging interleaves 4 gram rows per SBUF partition so each 2 MiB
store is one contiguous 16 KiB run per partition (fewest HWDGE
descriptors), alternating between the SP and ACT descriptor engines.
"""
import os
import numpy as np

# per-core location starts (each core computes 13 consecutive locations;
# odd cores' 13th overlaps the next core, core 7's 13th is garbage)
_LO = [0, 13, 25, 38, 50, 63, 75, 88]
_CNT = [13, 12, 13, 12, 13, 12, 13, 12]
# conv2 channel-slice starts; delta_k = 1024*lo_k - 100*ch_lo_k is 0 (even k)
# or 12 (odd k)
_CH_LO = [0, 133, 256, 389, 512, 645, 768, 901]
_NSL = 136  # channels per conv2 slice (covers 12 + 13*1024 flat elements)

_CACHE = {}


def _build_nc():
    from concourse import bacc, tile, mybir

    f32 = mybir.dt.float32
    f32r = mybir.dt.float32r
    AF = mybir.ActivationFunctionType

    nc = bacc.Bacc("TRN2", target_bir_lowering=False, debug=False)

    inp_d = nc.dram_tensor("inp", [128, 4, 27, 25], f32r, kind="ExternalInput")
    w1_d = nc.dram_tensor("w1t", [128, 4, 9, 512], f32r, kind="ExternalInput")
    b1_d = nc.dram_tensor("b1t", [128, 4], f32, kind="ExternalInput")
    w2_d = nc.dram_tensor("w2t", [128, 4, 9, _NSL], f32, kind="ExternalInput")
    b2_d = nc.dram_tensor("b2t", [128, 2], f32, kind="ExternalInput")
    sel_d = nc.dram_tensor("selt", [16, 2], f32, kind="ExternalInput")
    id_d = nc.dram_tensor("ident", [16, 16], f32, kind="ExternalInput")
    gp_d = nc.dram_tensor("gpart", [13, 1024, 1024], f32, kind="ExternalOutput")
    scr_d = nc.dram_tensor("scratch", [137, 100], f32)

    with tile.TileContext(nc) as tc:
        with tc.tile_pool(name="consts", bufs=1) as cp, \
             tc.tile_pool(name="work", bufs=1) as wp:

            w2sb = cp.tile([128, 4, 9, _NSL], f32)
            b1sb = cp.tile([128, 4], f32)
            b2sb = cp.tile([128, 2], f32)
            selsb = cp.tile([16, 2], f32)
            idsb = cp.tile([16, 16], f32)
            onesb = cp.tile([1, 128], f32)

            nc.sync.dma_start(out=w2sb[:], in_=w2_d.ap())
            nc.sync.dma_start(out=b1sb[:], in_=b1_d.ap())
            nc.sync.dma_start(out=b2sb[:], in_=b2_d.ap())
            nc.sync.dma_start(out=selsb[:], in_=sel_d.ap())
            nc.sync.dma_start(out=idsb[:], in_=id_d.ap())
            nc.vector.memset(onesb[:], 1.0)

            # ---- conv1: (512,27,25)->(512,23,23), fp32r, replicated ----
            convp = tc.alloc_tile_pool(name="convp", bufs=1)
            ps1 = tc.alloc_tile_pool(name="ps1", bufs=1, space="PSUM")
            insb = convp.tile([128, 4, 27, 25], f32r)
            w1sb = convp.tile([128, 4, 9, 512], f32r)
            for c in range(4):
                nc.sync.dma_start(out=insb[:, c], in_=inp_d.ap()[:, c])
                nc.sync.dma_start(out=w1sb[:, c], in_=w1_d.ap()[:, c])

            c1sb = wp.tile([128, 4, 24, 24], f32)
            nc.vector.memset(c1sb[:, :, 23:24, :], 0.0)
            nc.vector.memset(c1sb[:, :, :, 23:24], 0.0)
            # 8 live accumulation groups, ci-chunk outer so compute overlaps
            # the streaming w1 chunk loads
            c1groups = [(m, r0, nr) for m in range(4)
                        for (r0, nr) in [(0, 12), (12, 11)]]
            c1ps = [ps1.tile([128, 300], f32, tag=f"c1p{gi}",
                             name=f"c1ps{gi}") for gi in range(8)]
            for c in range(4):
                flat_c = insb[:, c].rearrange("p a b -> p (a b)")
                for gi, (m, r0, nr) in enumerate(c1groups):
                    for t in range(9):
                        dy, dx = t // 3, t % 3
                        s0 = (r0 + dy) * 25 + dx
                        nc.tensor.matmul(
                            c1ps[gi][:],
                            w1sb[:, c, t, m * 128:(m + 1) * 128],
                            flat_c[:, s0:s0 + 300],
                            start=(c == 0 and t == 0),
                            stop=(c == 3 and t == 8),
                        )
            for gi, (m, r0, nr) in enumerate(c1groups):
                nc.scalar.activation(
                    out=c1sb[:, m, r0:r0 + nr, 0:23],
                    in_=c1ps[gi][:, 0:300].rearrange("p (a b) -> p a b", b=25)[:, 0:nr, 0:23],
                    func=AF.Relu,
                    bias=b1sb[:, m:m + 1],
                )

            # ---- maxpool 2x2 ceil -> (512,12,12) (pad cells are 0, relu>=0) ----
            colmax = wp.tile([128, 4, 24, 12], f32)
            cpair = c1sb[:].rearrange("p c r (w two) -> p c r w two", two=2)
            nc.vector.tensor_max(colmax[:], cpair[:, :, :, :, 0],
                                 cpair[:, :, :, :, 1])
            pooled = wp.tile([128, 4, 12, 12], f32)
            rpair = colmax[:].rearrange("p c (r two) w -> p c r two w", two=2)
            nc.vector.tensor_max(pooled[:], rpair[:, :, :, 0, :],
                                 rpair[:, :, :, 1, :])

            # ---- conv2 slice: 136 output channels, exact fp32 ----
            ps1.release()
            ps2 = tc.alloc_tile_pool(name="ps2", bufs=2, space="PSUM")
            c2sb = wp.tile([128, 2, 100], f32)
            for m, (mo, mw) in enumerate([(0, 128), (128, 8)]):
                ps = ps2.tile([128, 100], f32, tag="c2p")
                for c in range(4):
                    for t in range(9):
                        dy, dx = t // 3, t % 3
                        nc.tensor.matmul(
                            ps[0:mw, :],
                            w2sb[:, c, t, mo:mo + mw],
                            pooled[:, c, dy:dy + 10, dx:dx + 10],
                            start=(c == 0 and t == 0),
                            stop=(c == 3 and t == 8),
                        )
                nc.scalar.activation(
                    out=c2sb[0:mw, m, :],
                    in_=ps[0:mw, :],
                    func=AF.Relu,
                    bias=b2sb[0:mw, m:m + 1],
                )

            # ---- flat view via DRAM scratch roundtrip ----
            nc.sync.dma_start(out=scr_d.ap()[0:128, :], in_=c2sb[:, 0, :])
            nc.sync.dma_start(out=scr_d.ap()[128:136, :], in_=c2sb[0:8, 1, :])

            flat = scr_d.ap().rearrange("a b -> (a b)")
            T0 = wp.tile([13, 1024], f32)
            T12 = wp.tile([13, 1024], f32)
            nc.scalar.dma_start(
                out=T0[:], in_=flat[0:13312].rearrange("(p i) -> p i", i=1024))
            nc.scalar.dma_start(
                out=T12[:], in_=flat[12:13324].rearrange("(p i) -> p i", i=1024))
            Tsel = wp.tile([13, 1024], f32)
            nc.vector.tensor_scalar_mul(Tsel[:], T0[:], selsb[0:13, 0:1])
            nc.vector.scalar_tensor_tensor(
                out=Tsel[:], in0=T12[:], scalar=selsb[0:13, 1:2], in1=Tsel[:],
                op0=mybir.AluOpType.mult, op1=mybir.AluOpType.add,
            )

            ps2.release()
            convp.release()

            vp = tc.alloc_tile_pool(name="vrow", bufs=6)
            bp_pool = tc.alloc_tile_pool(name="bcast", bufs=3)
            sp = tc.alloc_tile_pool(name="stage", bufs=6)
            psT = tc.alloc_tile_pool(name="psT", bufs=2, space="PSUM")
            psB = tc.alloc_tile_pool(name="psB", bufs=3, space="PSUM")

            # tcol[p, 4u+x, l] = v_l[512u + 4p + x]  (4-row interleave)
            tcol = wp.tile([128, 8, 16], f32)
            for u in range(2):
                lhs4 = Tsel[:, 512 * u:512 * (u + 1)].rearrange(
                    "l (m four) -> l four m", four=4)
                for x in range(4):
                    pst = psT.tile([128, 16], f32, tag="tc")
                    nc.tensor.matmul(
                        pst[:, 0:13], lhs4[:, x, :], idsb[0:13, 0:13],
                        start=True, stop=True,
                    )
                    nc.vector.tensor_copy(tcol[:, 4 * u + x, 0:13],
                                          pst[:, 0:13])

            # ---- Gram outer products, exact fp32 on DVE/ACT ----
            for li in range(13):
                vrow = vp.tile([1, 1024], f32, tag="vrow")
                nc.sync.dma_start(out=vrow[:], in_=Tsel[li:li + 1, :])
                bp = psB.tile([128, 1024], f32, tag="bc")
                nc.tensor.matmul(bp[:, 0:512], onesb[:],
                                 vrow[0:1, 0:512],
                                 start=True, stop=True)
                nc.tensor.matmul(bp[:, 512:1024], onesb[:],
                                 vrow[0:1, 512:1024],
                                 start=True, stop=True)
                bc = bp_pool.tile([128, 1024], f32, tag="bcs")
                nc.vector.tensor_copy(bc[:, 0:512], bp[:, 0:512])
                nc.scalar.activation(bc[:, 512:1024], bp[:, 512:1024],
                                     func=AF.Copy)
                for u in range(2):
                    st = sp.tile([128, 4096], f32, tag="st")
                    for x in range(4):
                        col = tcol[:, 4 * u + x, li:li + 1]
                        dve = (x % 2 == 0) or (u == 1 and x == 3)
                        if dve:
                            nc.vector.tensor_scalar_mul(
                                st[:, x * 1024:(x + 1) * 1024], bc[:], col)
                        else:
                            nc.scalar.activation(
                                st[:, x * 1024:(x + 1) * 1024], bc[:],
                                func=AF.Copy, scale=col)
                    dst = gp_d.ap()[li, 512 * u:512 * (u + 1), :].rearrange(
                        "(q four) f -> q (four f)", four=4)
                    if u == 0:
                        nc.sync.dma_start(out=dst, in_=st[:])
                    else:
                        nc.scalar.dma_start(out=dst, in_=st[:])
            psB.release()
            psT.release()
            sp.release()
            bp_pool.release()
            vp.release()

    nc.compile()
    return nc


def _get_nc():
    if "nc" not in _CACHE:
        _CACHE["nc"] = _build_nc()
    return _CACHE["nc"]


def _host_prep(input, w1, b1, w2, b2):
    x = np.asarray(input, np.float32).reshape(512, 25, 25)
    w1 = np.asarray(w1, np.float32)
    w2 = np.asarray(w2, np.float32)
    b1 = np.asarray(b1, np.float32)
    b2 = np.asarray(b2, np.float32)

    inp = np.zeros((4, 128, 27, 25), np.float32)
    inp[:, :, :25, :] = x.reshape(4, 128, 25, 25)
    inp = np.ascontiguousarray(inp.transpose(1, 0, 2, 3))

    w1t = w1.reshape(512, 512, 9).transpose(1, 2, 0)          # [ci, 9, co]
    w1t = np.ascontiguousarray(
        w1t.reshape(4, 128, 9, 512).transpose(1, 0, 2, 3))    # [128,4,9,512]
    b1t = np.ascontiguousarray(b1.reshape(4, 128).T)

    ident = np.eye(16, dtype=np.float32)

    common = {"inp": inp, "w1t": w1t, "b1t": b1t, "ident": ident}
    in_maps = []
    for k in range(8):
        ch = _CH_LO[k]
        nval = min(1024, ch + _NSL) - ch
        wsl = np.zeros((_NSL, 512, 9), np.float32)
        wsl[:nval] = w2.reshape(1024, 512, 9)[ch:ch + nval]
        w2t = wsl.transpose(1, 2, 0)                           # [512,9,136]
        w2t = np.ascontiguousarray(
            w2t.reshape(4, 128, 9, _NSL).transpose(1, 0, 2, 3))
        bsl = np.zeros(256, np.float32)
        bsl[:nval] = b2[ch:ch + nval]
        b2t = np.ascontiguousarray(bsl.reshape(2, 128).T)
        delta_is_12 = (1024 * _LO[k] - 100 * ch) == 12
        selt = np.zeros((16, 2), np.float32)
        selt[:, 0] = 0.0 if delta_is_12 else 1.0
        selt[:, 1] = 1.0 if delta_is_12 else 0.0
        in_maps.append({**common, "w2t": w2t, "b2t": b2t, "selt": selt})
    return in_maps


def kernel(input, w1, b1, w2, b2):
    from concourse import bass_utils

    nc = _get_nc()
    in_maps = _host_prep(input, w1, b1, w2, b2)

    prof_dir = os.environ.get("GRAM_KERNEL_PROFILE_DIR")
    if prof_dir:
        from trn_agent_boot.trn_boot import _ntff_profile_via_ctypes
        hook = _ntff_profile_via_ctypes('/opt/axon/libaxon_pjrt.so')
        with hook(prof_dir, [0]):
            res = bass_utils.run_bass_kernel_spmd(
                nc, in_maps, core_ids=list(range(8)))
    else:
        res = bass_utils.run_bass_kernel_spmd(
            nc, in_maps, core_ids=list(range(8)))

    out = np.empty((100, 1024, 1024), np.float32)
    for k in range(8):
        out[_LO[k]:_LO[k] + _CNT[k]] = res.results[k]["gpart"][:_CNT[k]]
    return out



# revision 2
# speedup vs baseline: 1.1400x; 1.1400x over previous
"""Trainium2 Bass kernel for nn_AttentionBranch: conv->relu->maxpool->conv->relu
followed by per-location rank-1 Gram outer products (100, 1024, 1024).

Sharding: the 100-location Gram axis is split across 8 NeuronCores
(13/12 locations per core). The conv backbone is replicated (conv1) /
channel-sliced to each core's needed 136-channel window (conv2), so no
collectives are required. The row-major .view(100, 1024) of the conv2
output is realised through a tiny DRAM scratch roundtrip.

Numerics: conv1/conv2 matmuls run in fp16 (single-pass on TensorE,
fp32 PSUM accumulation, ~1e-3 rel), and the Gram products exactly in
fp32 on VectorE/ScalarE (tensor_scalar against a PE-broadcast row).

The T0/T12 row select (the per-core flat-offset delta of 0 or 12) is
folded into the broadcast / tcol matmuls as accumulation pairs scaled
by s0/s1, so no select pass sits on the first-store critical path.

Output staging interleaves 4 gram rows per SBUF partition so each 2 MiB
store is one contiguous 16 KiB run per partition (fewest HWDGE
descriptors), alternating between the SP and ACT descriptor engines.
"""
import os
import numpy as np

# per-core location starts (each core computes 13 consecutive locations;
# odd cores' 13th overlaps the next core, core 7's 13th is garbage)
_LO = [0, 13, 25, 38, 50, 63, 75, 88]
_CNT = [13, 12, 13, 12, 13, 12, 13, 12]
# conv2 channel-slice starts; delta_k = 1024*lo_k - 100*ch_lo_k is 0 (even k)
# or 12 (odd k)
_CH_LO = [0, 133, 256, 389, 512, 645, 768, 901]
_NSL = 136  # channels per conv2 slice (covers 12 + 13*1024 flat elements)

_CACHE = {}


def _build_nc():
    from concourse import bacc, tile, mybir

    f32 = mybir.dt.float32
    f16 = mybir.dt.float16
    AF = mybir.ActivationFunctionType

    nc = bacc.Bacc("TRN2", target_bir_lowering=False, debug=False)

    inp_d = nc.dram_tensor("inp", [128, 4, 27, 25], f16, kind="ExternalInput")
    w1_d = nc.dram_tensor("w1t", [128, 4, 9, 512], f16, kind="ExternalInput")
    b1_d = nc.dram_tensor("b1t", [128, 4], f32, kind="ExternalInput")
    w2_d = nc.dram_tensor("w2t", [128, 4, 9, _NSL], f16, kind="ExternalInput")
    b2_d = nc.dram_tensor("b2t", [128, 2], f32, kind="ExternalInput")
    sel_d = nc.dram_tensor("selt", [16, 2], f32, kind="ExternalInput")
    id_d = nc.dram_tensor("ident", [16, 16], f32, kind="ExternalInput")
    gp_d = nc.dram_tensor("gpart", [13, 1024, 1024], f32, kind="ExternalOutput")
    scr_d = nc.dram_tensor("scratch", [137, 100], f32)

    with tile.TileContext(nc) as tc:
        with tc.tile_pool(name="consts", bufs=1) as cp, \
             tc.tile_pool(name="work", bufs=1) as wp:

            convp = tc.alloc_tile_pool(name="convp", bufs=1)
            ps1 = tc.alloc_tile_pool(name="ps1", bufs=1, space="PSUM")

            # ---- conv1-critical loads first, alternating HWDGE rings ----
            insb = convp.tile([128, 4, 27, 25], f16)
            w1sb = convp.tile([128, 4, 9, 512], f16)
            for c in range(4):
                eng = nc.sync if c % 2 == 0 else nc.scalar
                eng.dma_start(out=insb[:, c], in_=inp_d.ap()[:, c])
                eng.dma_start(out=w1sb[:, c], in_=w1_d.ap()[:, c])

            w2sb = cp.tile([128, 4, 9, _NSL], f16)
            b1sb = cp.tile([128, 4], f32)
            b2sb = cp.tile([128, 2], f32)
            selsb = cp.tile([16, 2], f32)
            idsb = cp.tile([16, 16], f32)
            onesb = cp.tile([1, 128], f32)
            ones_s0 = cp.tile([1, 128], f32)
            ones_s1 = cp.tile([1, 128], f32)
            ids_s0 = cp.tile([16, 16], f32)
            ids_s1 = cp.tile([16, 16], f32)

            nc.scalar.dma_start(out=b1sb[:], in_=b1_d.ap())
            nc.scalar.dma_start(out=selsb[:], in_=sel_d.ap())
            nc.scalar.dma_start(out=idsb[:], in_=id_d.ap())
            nc.scalar.dma_start(out=w2sb[:], in_=w2_d.ap())
            nc.scalar.dma_start(out=b2sb[:], in_=b2_d.ap())

            # select scalars folded into matmul operands (s0/s1 is 1/0 or 0/1)
            nc.vector.memset(onesb[:], 1.0)
            nc.vector.tensor_scalar_mul(ones_s0[:], onesb[:], selsb[0:1, 0:1])
            nc.vector.tensor_scalar_mul(ones_s1[:], onesb[:], selsb[0:1, 1:2])
            nc.vector.tensor_scalar_mul(ids_s0[0:13, 0:13], idsb[0:13, 0:13],
                                        selsb[0:13, 0:1])
            nc.vector.tensor_scalar_mul(ids_s1[0:13, 0:13], idsb[0:13, 0:13],
                                        selsb[0:13, 1:2])

            # ---- conv1: (512,27,25)->(512,23,23), fp16, replicated ----
            c1sb = convp.tile([128, 4, 24, 24], f16)
            nc.vector.memset(c1sb[:, :, 23:24, :], 0.0)
            nc.vector.memset(c1sb[:, :, :, 23:24], 0.0)
            # 8 live accumulation groups, ci-chunk outer so compute overlaps
            # the streaming w1 chunk loads
            c1groups = [(m, r0, nr) for m in range(4)
                        for (r0, nr) in [(0, 12), (12, 11)]]
            c1ps = [ps1.tile([128, 300], f32, tag=f"c1p{gi}",
                             name=f"c1ps{gi}") for gi in range(8)]
            for c in range(4):
                flat_c = insb[:, c].rearrange("p a b -> p (a b)")
                for gi, (m, r0, nr) in enumerate(c1groups):
                    for t in range(9):
                        dy, dx = t // 3, t % 3
                        s0 = (r0 + dy) * 25 + dx
                        nc.tensor.matmul(
                            c1ps[gi][:],
                            w1sb[:, c, t, m * 128:(m + 1) * 128],
                            flat_c[:, s0:s0 + 300],
                            start=(c == 0 and t == 0),
                            stop=(c == 3 and t == 8),
                        )
            for gi, (m, r0, nr) in enumerate(c1groups):
                nc.scalar.activation(
                    out=c1sb[:, m, r0:r0 + nr, 0:23],
                    in_=c1ps[gi][:, 0:300].rearrange("p (a b) -> p a b", b=25)[:, 0:nr, 0:23],
                    func=AF.Relu,
                    bias=b1sb[:, m:m + 1],
                )

            # ---- maxpool 2x2 ceil -> (512,12,12) (pad cells are 0, relu>=0) ----
            colmax = convp.tile([128, 4, 24, 12], f16)
            cpair = c1sb[:].rearrange("p c r (w two) -> p c r w two", two=2)
            nc.vector.tensor_max(colmax[:], cpair[:, :, :, :, 0],
                                 cpair[:, :, :, :, 1])
            pooled = convp.tile([128, 4, 12, 12], f16)
            rpair = colmax[:].rearrange("p c (r two) w -> p c r two w", two=2)
            nc.vector.tensor_max(pooled[:], rpair[:, :, :, 0, :],
                                 rpair[:, :, :, 1, :])

            # ---- conv2 slice: 136 output channels, fp16 ----
            ps1.release()
            ps2 = tc.alloc_tile_pool(name="ps2", bufs=2, space="PSUM")
            c2sb = wp.tile([128, 2, 100], f32)
            for m, (mo, mw) in enumerate([(0, 128), (128, 8)]):
                ps = ps2.tile([128, 100], f32, tag="c2p")
                for c in range(4):
                    for t in range(9):
                        dy, dx = t // 3, t % 3
                        nc.tensor.matmul(
                            ps[0:mw, :],
                            w2sb[:, c, t, mo:mo + mw],
                            pooled[:, c, dy:dy + 10, dx:dx + 10],
                            start=(c == 0 and t == 0),
                            stop=(c == 3 and t == 8),
                        )
                nc.scalar.activation(
                    out=c2sb[0:mw, m, :],
                    in_=ps[0:mw, :],
                    func=AF.Relu,
                    bias=b2sb[0:mw, m:m + 1],
                )
                # flat view via DRAM scratch: store each chunk as it finishes
                if m == 0:
                    nc.sync.dma_start(out=scr_d.ap()[0:128, :],
                                      in_=c2sb[:, 0, :])
                else:
                    nc.sync.dma_start(out=scr_d.ap()[128:136, :],
                                      in_=c2sb[0:8, 1, :])

            flat = scr_d.ap().rearrange("a b -> (a b)")
            # all 13 rows (both delta variants) on partition 0 in one load
            vall = wp.tile([1, 13324], f32)
            nc.sync.dma_start(
                out=vall[:], in_=flat[0:13324].rearrange("(p i) -> p i", p=1))
            # row-major [13, 1024] views for the tcol transposes
            T0 = wp.tile([13, 1024], f32)
            T12 = wp.tile([13, 1024], f32)
            nc.scalar.dma_start(
                out=T0[:], in_=flat[0:13312].rearrange("(p i) -> p i", i=1024))
            nc.scalar.dma_start(
                out=T12[:], in_=flat[12:13324].rearrange("(p i) -> p i", i=1024))

            ps2.release()
            convp.release()

            bp_pool = tc.alloc_tile_pool(name="bcast", bufs=3)
            sp = tc.alloc_tile_pool(name="stage", bufs=6)
            psT = tc.alloc_tile_pool(name="psT", bufs=2, space="PSUM")
            psB = tc.alloc_tile_pool(name="psB", bufs=3, space="PSUM")

            tcol = wp.tile([128, 8, 16], f32)

            def build_tcol():
                # tcol[p, 4u+x, l] = v_l[512u + 4p + x]  (4-row interleave),
                # with the T0/T12 select accumulated in PSUM
                for u in range(2):
                    l4_0 = T0[:, 512 * u:512 * (u + 1)].rearrange(
                        "l (m four) -> l four m", four=4)
                    l4_12 = T12[:, 512 * u:512 * (u + 1)].rearrange(
                        "l (m four) -> l four m", four=4)
                    for x in range(4):
                        pst = psT.tile([128, 16], f32, tag="tc")
                        nc.tensor.matmul(pst[:, 0:13], l4_0[:, x, :],
                                         ids_s0[0:13, 0:13],
                                         start=True, stop=False)
                        nc.tensor.matmul(pst[:, 0:13], l4_12[:, x, :],
                                         ids_s1[0:13, 0:13],
                                         start=False, stop=True)
                        nc.vector.tensor_copy(tcol[:, 4 * u + x, 0:13],
                                              pst[:, 0:13])

            # ---- Gram outer products, exact fp32 on DVE/ACT ----
            for li in range(13):
                # broadcast row li to 128 partitions; select via accumulation
                bp = psB.tile([128, 1024], f32, tag="bc")
                for h in range(2):
                    o0 = 1024 * li + 512 * h
                    nc.tensor.matmul(bp[:, 512 * h:512 * (h + 1)], ones_s0[:],
                                     vall[0:1, o0:o0 + 512],
                                     start=True, stop=False)
                    nc.tensor.matmul(bp[:, 512 * h:512 * (h + 1)], ones_s1[:],
                                     vall[0:1, o0 + 12:o0 + 524],
                                     start=False, stop=True)
                if li == 0:
                    build_tcol()
                bc = bp_pool.tile([128, 1024], f32, tag="bcs")
                nc.vector.tensor_copy(bc[:, 0:512], bp[:, 0:512])
                nc.scalar.activation(bc[:, 512:1024], bp[:, 512:1024],
                                     func=AF.Copy)
                for u in range(2):
                    st = sp.tile([128, 4096], f32, tag="st")
                    for x in range(4):
                        col = tcol[:, 4 * u + x, li:li + 1]
                        dve = (x % 2 == 0) or (u == 1 and x == 3)
                        if dve:
                            nc.vector.tensor_scalar_mul(
                                st[:, x * 1024:(x + 1) * 1024], bc[:], col)
                        else:
                            nc.scalar.activation(
                                st[:, x * 1024:(x + 1) * 1024], bc[:],
                                func=AF.Copy, scale=col)
                    dst = gp_d.ap()[li, 512 * u:512 * (u + 1), :].rearrange(
                        "(q four) f -> q (four f)", four=4)
                    if u == 0:
                        nc.sync.dma_start(out=dst, in_=st[:])
                    else:
                        nc.scalar.dma_start(out=dst, in_=st[:])
            psB.release()
            psT.release()
            sp.release()
            bp_pool.release()

    nc.compile()
    return nc


def _get_nc():
    if "nc" not in _CACHE:
        _CACHE["nc"] = _build_nc()
    return _CACHE["nc"]


def _host_prep(input, w1, b1, w2, b2):
    x = np.asarray(input, np.float32).reshape(512, 25, 25)
    w1 = np.asarray(w1, np.float32)
    w2 = np.asarray(w2, np.float32)
    b1 = np.asarray(b1, np.float32)
    b2 = np.asarray(b2, np.float32)

    inp = np.zeros((4, 128, 27, 25), np.float32)
    inp[:, :, :25, :] = x.reshape(4, 128, 25, 25)
    inp = np.ascontiguousarray(inp.transpose(1, 0, 2, 3)).astype(np.float16)

    w1t = w1.reshape(512, 512, 9).transpose(1, 2, 0)          # [ci, 9, co]
    w1t = np.ascontiguousarray(
        w1t.reshape(4, 128, 9, 512).transpose(1, 0, 2, 3)).astype(np.float16)
    b1t = np.ascontiguousarray(b1.reshape(4, 128).T)

    ident = np.eye(16, dtype=np.float32)

    common = {"inp": inp, "w1t": w1t, "b1t": b1t, "ident": ident}
    in_maps = []
    for k in range(8):
        ch = _CH_LO[k]
        nval = min(1024, ch + _NSL) - ch
        wsl = np.zeros((_NSL, 512, 9), np.float32)
        wsl[:nval] = w2.reshape(1024, 512, 9)[ch:ch + nval]
        w2t = wsl.transpose(1, 2, 0)                           # [512,9,136]
        w2t = np.ascontiguousarray(
            w2t.reshape(4, 128, 9, _NSL).transpose(1, 0, 2, 3)).astype(
                np.float16)
        bsl = np.zeros(256, np.float32)
        bsl[:nval] = b2[ch:ch + nval]
        b2t = np.ascontiguousarray(bsl.reshape(2, 128).T)
        delta_is_12 = (1024 * _LO[k] - 100 * ch) == 12
        selt = np.zeros((16, 2), np.float32)
        selt[:, 0] = 0.0 if delta_is_12 else 1.0
        selt[:, 1] = 1.0 if delta_is_12 else 0.0
        in_maps.append({**common, "w2t": w2t, "b2t": b2t, "selt": selt})
    return in_maps


def kernel(input, w1, b1, w2, b2):
    from concourse import bass_utils

    nc = _get_nc()
    in_maps = _host_prep(input, w1, b1, w2, b2)

    prof_dir = os.environ.get("GRAM_KERNEL_PROFILE_DIR")
    if prof_dir:
        from trn_agent_boot.trn_boot import _ntff_profile_via_ctypes
        hook = _ntff_profile_via_ctypes('/opt/axon/libaxon_pjrt.so')
        with hook(prof_dir, [0]):
            res = bass_utils.run_bass_kernel_spmd(
                nc, in_maps, core_ids=list(range(8)))
    else:
        res = bass_utils.run_bass_kernel_spmd(
            nc, in_maps, core_ids=list(range(8)))

    out = np.empty((100, 1024, 1024), np.float32)
    for k in range(8):
        out[_LO[k]:_LO[k] + _CNT[k]] = res.results[k]["gpart"][:_CNT[k]]
    return out
